# revision 8
# baseline (speedup 1.0000x reference)
"""CrossAttention3D Trainium2 kernel (restructured).

Full inputs in, full output out. Sharding: data-parallel over batch (2) x
query-token shards (4) = 8 NeuronCores; each core runs 1024 queries against
all 4096 keys.

Host-side algebraic folding removes two of the four projections:
  scores = src^T (Wq^T Wk / sqrt(C)) tgt  -> one folded K-projection, src is
  used directly as the query matrix. The per-key bias term beta_k =
  (bq . Wk tgt_k + bq.bk)/sqrt(C) is folded into exp (per-query terms are
  softmax-invariant and dropped exactly).
  out = wo @ (attn V) + .. -> V' = (wo wv) tgt, so the PV contraction directly
  produces output channels; wo@bv + bo is added on the host.

Attention layout: scores st[k,1024q] per 128-key tile (k on partitions); exp
tiles feed PV as matmul *weights* (128-query chunks) against V'^T tiles
augmented with a ones column [128k, 128c+1], accumulating opsum[q, c+1] in
PSUM across all 32 key tiles — the softmax denominator falls out of the same
matmuls as channel 128, eliminating the entire DVE accumulation chain. The
[q, c] output is transposed on the host.

exp runs split across two engines: Act (hardware Exp, bias=beta) and DVE via
two chained custom-DVE ops computing exp(x+beta) ~ [q2(q1(x))]^4 where q1,q2
are shared-slope factored cubics (minimax-fitted, 2.3e-3 max rel err over
|x|<=8.6); beta enters through q1's per-partition scalar operands.
"""

import numpy as np

import concourse.bass as bass
import concourse.mybir as mybir
import concourse.tile as tile
from concourse.bass_utils import run_bass_kernel_spmd
from concourse.vector_clock import ScopedClock

F32 = mybir.dt.float32
F16 = mybir.dt.float16

B, C, D, H, W = 2, 128, 16, 16, 16
N = D * H * W          # 4096 target tokens
NCORES = 8
QSHARDS = NCORES // B  # 4 query shards per batch
NQ = N // QSHARDS      # 1024 query tokens per core
KT = N // 128          # 32 key tiles
QC = NQ // 128         # 8 query chunks of 128
AF = mybir.ActivationFunctionType
OP = mybir.AluOpType

# exp(x) ~ [q2(q1(x))]^4, q_i(x) = (sq(s*x+b)+c)*(s*x+w)  (shared-slope cubics)
S1, B1, C1, W1 = 0.05194748781283326, 0.2171248992897139, 1.4819305023704588, 0.56936452908816459
S2, B2, C2, W2 = 1.0244462795933973, -0.098833807649857053, 0.29508885935180756, 0.19029542731487301

# Custom DVE ops do not compile on this walrus build ("ISA wrong length" in
# codegen even for production ops), so exp runs entirely on the Act engine.
DVE_TILES = frozenset()


# --- walrus sync-wait workarounds (unchanged from baseline) ------------------

def _patched_drain_and_barrier(self, tick_clock, wait_clock):
    # This walrus build caps sync-waits per instruction; the stock TileContext
    # exit drain carries one wait per processor lane (>4 in this kernel).
    # Split the waits into single-wait SP instructions before the drain.
    nc = self.nc
    probe = nc.sync.nop()
    wait_clock.add_sem_waits(probe.ins, ScopedClock({None: tick_clock.global_clock}))
    si = probe.ins.sync_info
    waits = list(si.on_wait) if si and si.on_wait else []
    if si:
        si.on_wait = []
        probe.ins.sync_info = si
    by_name = {h.name: h for h in self.sems.allocated().values()}
    opmap = {"sem-ge-imm": "sem-ge", "sem-eq-imm": "sem-eq"}
    for wv in waits:
        nc.sync.wait_op(by_name[wv.ant_name], wv.wait_value, opmap.get(wv.wait_mode, "sem-ge"))
    nc.sync.drain()
    nc.all_engine_barrier()
    popped = nc._tile_sem_poison_stack.pop()
    assert popped is self._sem_poison
    nc.clear_and_free_semaphores(list(self.sems.allocated().values()))


tile.TileContext._drain_and_barrier = _patched_drain_and_barrier


def _split_excess_waits(nc, cap=1, evsem_cap=2):
    # This walrus build rejects instructions carrying more than ~1 sync wait
    # (Tile targets a newer walrus that packs several). Hoist excess waits
    # onto dedicated InstEventSemaphore instructions just before the
    # over-subscribed instruction, on the same engine stream.
    for fn in nc.m.functions:
        for bb in fn.blocks:
            out = []
            for inst in bb.instructions:
                si = inst.sync_info
                waits = list(si.on_wait) if si and si.on_wait else []
                limit = (
                    evsem_cap
                    if isinstance(inst, (mybir.InstEventSemaphore, mybir.InstDrain))
                    else cap
                )
                if len(waits) > limit:
                    excess, keep = waits[:-limit], waits[-limit:]
                    for i in range(0, len(excess), evsem_cap):
                        ev = mybir.InstEventSemaphore(
                            name=nc.get_next_instruction_name(),
                            engine=inst.engine,
                            ins=[],
                            outs=[],
                            sync_info=mybir.SyncInfo(
                                on_wait=excess[i : i + evsem_cap], on_update=[]
                            ),
                        )
                        nc.register_instruction(ev)
                        out.append(ev)
                    si.on_wait = keep
                    inst.sync_info = si
                out.append(inst)
            bb.instructions[:] = out


# --- kernel ------------------------------------------------------------------

def build_bass():
    nc = bass.Bass("TRN2", target_bir_lowering=False, debug=False)

    srcq = nc.dram_tensor("srcq", [C, NQ], F16, kind="ExternalInput")
    tgt = nc.dram_tensor("tgt", [C, N], F16, kind="ExternalInput")
    mw = nc.dram_tensor("mw", [C, 2, C], F16, kind="ExternalInput")   # M^T | Wvo^T
    bet3 = nc.dram_tensor("bet3", [C, 3, KT], F32, kind="ExternalInput")  # beta|b1'|w1'
    out = nc.dram_tensor("out", [C, QC, C], F16, kind="ExternalOutput")   # [qpart, chunk, co]

    with tile.TileContext(nc) as tc:
        with (
            tc.tile_pool(name="consts", bufs=1) as consts,
            tc.tile_pool(name="big", bufs=1) as big,
            tc.tile_pool(name="ets", bufs=4) as ets,
            tc.tile_pool(name="psum", bufs=2, space="PSUM") as psum,
            tc.tile_pool(name="psum_pv", bufs=1, space="PSUM") as psum_pv,
        ):
            # ---- inputs on two HWDGE rings (SP + ACT); critical tensors first.
            mw_sb = consts.tile([C, 2, C], F16)
            mt_h, wvo_h = mw_sb[:, 0, :], mw_sb[:, 1, :]
            bet3_sb = consts.tile([C, 3, KT], F32)
            beta_sb = bet3_sb[:, 0, :]
            tgt_c = [big.tile([C, 1024], F16, name=f"tgt_c{j}") for j in range(4)]
            srcq_sb = big.tile([C, NQ], F16)

            nc.sync.dma_start(mw_sb[:], mw[:, :, :])
            nc.scalar.dma_start(tgt_c[0][:], tgt[:, 0:1024])
            nc.sync.dma_start(tgt_c[1][:], tgt[:, 1024:2048])
            nc.scalar.dma_start(bet3_sb[:], bet3[:, :, :])
            nc.sync.dma_start(srcq_sb[:], srcq[:, :])
            nc.scalar.dma_start(tgt_c[2][:], tgt[:, 2048:3072])
            nc.sync.dma_start(tgt_c[3][:], tgt[:, 3072:4096])

            warm_src = consts.tile([C, 512], F16)
            nc.gpsimd.memset(warm_src[:], 1.0)
            # V'^T tiles with denominator ones-column: [k-token, kt, c|1].
            # The column write is strided; gpsimd memset crashes the exec unit
            # on strided APs, DVE handles it.
            vta = big.tile([C, KT, C + 1], F16)
            nc.vector.memset(vta[:, :, C : C + 1], 1.0)

            # PE warm-up: dummy matmuls with no DMA deps ramp the HAM clock
            # while the input DMAs are in flight.
            for wi in range(7):
                warm_ps = psum.tile([C, 512], F32, tag="ps_big", bufs=2, name=f"warm_{wi}")
                nc.tensor.matmul(
                    warm_ps[:], warm_src[:, 0:128], warm_src[:], start=True, stop=True,
                )

            # ---- projections ----
            k_c = [big.tile([C, 1024], F16, name=f"k_c{j}") for j in range(4)]

            def emit_kproj(j):
                kp = psum.tile([C, 1024], F32, tag="ps_big", bufs=2, name=f"kp_{j}")
                for h in range(2):
                    nc.tensor.matmul(
                        kp[:, h * 512 : (h + 1) * 512],
                        mt_h,
                        tgt_c[j][:, h * 512 : (h + 1) * 512],
                        start=True,
                        stop=True,
                    )
                if j % 2 == 0:
                    nc.vector.tensor_copy(k_c[j][:], kp[:])
                else:
                    nc.scalar.activation(out=k_c[j][:], in_=kp[:], func=AF.Copy)

            def emit_vproj(g):
                # V'^T[m, c] = (Wvo @ tgt)^T for 8 m-tiles
                vp = psum.tile([C, 8, C], F32, tag="ps_big", bufs=2, name=f"vp_{g}")
                for i in range(8):
                    mt = g * 8 + i
                    nc.tensor.matmul(
                        vp[:, i, :],
                        tgt_c[mt // 8][:, (mt % 8) * C : (mt % 8 + 1) * C],
                        wvo_h,
                        start=True,
                        stop=True,
                    )
                if g % 2 == 0:
                    nc.scalar.activation(
                        out=vta[:, g * 8 : (g + 1) * 8, 0:C], in_=vp[:], func=AF.Copy
                    )
                else:
                    nc.vector.tensor_copy(vta[:, g * 8 : (g + 1) * 8, 0:C], vp[:])

            # ---- attention pipeline ----
            opsum_a = psum_pv.tile([C, 4, 256], F32, name="opsum_a")
            opsum_b = psum_pv.tile([C, 4, 256], F32, name="opsum_b")

            st_tiles = {}
            et_tiles = {}

            def emit_st(kt):
                st = psum.tile([C, NQ], F32, tag="ps_big", bufs=2, name=f"st_{kt}")
                for h in range(2):
                    nc.tensor.matmul(
                        st[:, h * 512 : (h + 1) * 512],
                        k_c[kt // 8][:, (kt % 8) * C : (kt % 8 + 1) * C],
                        srcq_sb[:, h * 512 : (h + 1) * 512],
                        start=True,
                        stop=True,
                    )
                st_tiles[kt] = st

            def emit_exp(kt):
                et = ets.tile([C, NQ], F16, tag="et", name=f"et_{kt}")
                st = st_tiles.pop(kt)
                nc.scalar.activation(
                    out=et[:], in_=st[:], func=AF.Exp,
                    bias=beta_sb[:, kt : kt + 1], scale=1.0,
                )
                et_tiles[kt] = et

            def emit_pv(kt):
                # Chunk pairs share a PSUM bank; a start=True matmul clears the
                # has_written bits for the WHOLE bank, so only the bank's first
                # region may use start. The bank-mate's first write then lands
                # on cleared bits and overwrites-and-sets, which is exactly the
                # accumulation-group opening it needs.
                et = et_tiles.pop(kt)
                for j in range(QC):
                    ops = opsum_a if j < 4 else opsum_b
                    nc.tensor.matmul(
                        ops[:, j % 4, 0 : C + 1],
                        et[:, j * 128 : (j + 1) * 128],
                        vta[:, kt, :],
                        start=(kt == 0 and j % 2 == 0),
                        stop=(kt == KT - 1),
                        skip_group_check=True,
                    )

            # ---- software-pipelined emission ----
            emit_kproj(0)
            emit_vproj(0)
            emit_kproj(1)
            emit_st(0)
            emit_exp(0)
            for kt in range(1, KT):
                if kt == 2:
                    emit_vproj(1)
                elif kt == 5:
                    emit_kproj(2)
                elif kt == 9:
                    emit_vproj(2)
                elif kt == 13:
                    emit_kproj(3)
                elif kt == 17:
                    emit_vproj(3)
                emit_st(kt)
                emit_exp(kt)
                emit_pv(kt - 1)
            emit_pv(KT - 1)

            # ---- epilogue: normalize [q, c] by the ones-column denominators
            recip_sb = big.tile([C, QC], F32)
            o_fin = big.tile([C, QC, C], F16)
            nc.vector.reciprocal(out=recip_sb[:, 0:4], in_=opsum_a[:, :, C])
            nc.vector.reciprocal(out=recip_sb[:, 4:8], in_=opsum_b[:, :, C])
            for j in range(QC):
                ops = opsum_a if j < 4 else opsum_b
                if j % 2 == 0:
                    nc.scalar.activation(
                        out=o_fin[:, j, :], in_=ops[:, j % 4, 0:C],
                        func=AF.Identity, scale=recip_sb[:, j : j + 1],
                    )
                else:
                    nc.vector.tensor_scalar(
                        out=o_fin[:, j, :], in0=ops[:, j % 4, 0:C],
                        scalar1=recip_sb[:, j : j + 1], scalar2=None, op0=OP.mult,
                    )
                if j == 3:
                    nc.sync.dma_start(out[:, 0:4, :], o_fin[:, 0:4, :])
            nc.scalar.dma_start(out[:, 4:8, :], o_fin[:, 4:8, :])

    _split_excess_waits(nc)
    return nc


_NC_CACHE = None


def _get_nc():
    global _NC_CACHE
    if _NC_CACHE is None:
        _NC_CACHE = build_bass()
    return _NC_CACHE


def make_in_maps(source, target, wq, bq, wk, bk, wv, bv, wo, bo):
    source = np.asarray(source, dtype=np.float32).reshape(B, C, N)
    target = np.asarray(target, dtype=np.float32).reshape(B, C, N)
    wq, wk, wv, wo = (np.asarray(x, np.float32) for x in (wq, wk, wv, wo))
    bq, bk, bv, bo = (np.asarray(x, np.float32) for x in (bq, bk, bv, bo))
    scale = np.float32(1.0 / np.sqrt(C))

    M = (wq.T @ wk) * scale                 # [c_src, c_tgt]
    Wvo = wo @ wv                            # [c_out, c_tgt]
    mw_v = np.ascontiguousarray(
        np.stack([M.T, Wvo.T], axis=1).astype(np.float16)
    )                                        # [ct, 2, c]

    src16 = source.astype(np.float16)
    tgt16 = target.astype(np.float16)

    in_maps = []
    bet3_b = []
    for b in range(B):
        beta = ((bq @ (wk @ target[b])) + np.float32(bq @ bk)) * scale  # [N]
        bvec = beta.reshape(KT, 128).T.astype(np.float32)               # [128, KT]
        b1p = np.float32(B1) + np.float32(S1) * bvec
        w1p = np.float32(W1) + np.float32(S1) * bvec
        bet3_b.append(np.ascontiguousarray(
            np.stack([bvec, b1p, w1p], axis=1).astype(np.float32)
        ))                                                              # [128, 3, KT]

    for core in range(NCORES):
        b, qs = divmod(core, QSHARDS)
        in_maps.append({
            "srcq": np.ascontiguousarray(src16[b, :, qs * NQ : (qs + 1) * NQ]),
            "tgt": np.ascontiguousarray(tgt16[b]),
            "mw": mw_v,
            "bet3": bet3_b[b],
        })
    return in_maps


def kernel(source, target, wq, bq, wk, bk, wv, bv, wo, bo):
    nc = _get_nc()
    in_maps = make_in_maps(source, target, wq, bq, wk, bk, wv, bv, wo, bo)
    res = run_bass_kernel_spmd(nc, in_maps, core_ids=list(range(NCORES)))
    bvo = (np.asarray(wo, np.float32) @ np.asarray(bv, np.float32)
           + np.asarray(bo, np.float32))                                # [C]
    full = np.empty((B, C, N), dtype=np.float32)
    for core in range(NCORES):
        b, qs = divmod(core, QSHARDS)
        o = np.asarray(res.results[core]["out"], np.float32)            # [p, j, co]
        full[b, :, qs * NQ : (qs + 1) * NQ] = (
            o.transpose(2, 1, 0).reshape(C, NQ) + bvo[:, None]
        )
    return full.reshape(B, C, D, H, W)


# revision 9
# speedup vs baseline: 1.0086x; 1.0086x over previous
"""CrossAttention3D Trainium2 kernel (restructured).

Full inputs in, full output out. Sharding: data-parallel over batch (2) x
query-token shards (4) = 8 NeuronCores; each core runs 1024 queries against
all 4096 keys.

Host-side algebraic folding removes two of the four projections:
  scores = src^T (Wq^T Wk / sqrt(C)) tgt  -> one folded K-projection, src is
  used directly as the query matrix. The per-key bias term beta_k =
  (bq . Wk tgt_k + bq.bk)/sqrt(C) is folded into exp (per-query terms are
  softmax-invariant and dropped exactly).
  out = wo @ (attn V) + .. -> V' = (wo wv) tgt, so the PV contraction directly
  produces output channels; wo@bv + bo is added on the host.

Attention layout: scores st[k,1024q] per 128-key tile (k on partitions); exp
tiles feed PV as matmul *weights* (128-query chunks) against V'^T tiles
augmented with a ones column [128k, 128c+1], accumulating opsum[q, c+1] in
PSUM across all 32 key tiles — the softmax denominator falls out of the same
matmuls as channel 128, eliminating the entire DVE accumulation chain. The
[q, c] output is transposed on the host.

exp runs split across two engines: Act (hardware Exp, bias=beta) and DVE via
two chained custom-DVE ops computing exp(x+beta) ~ [q2(q1(x))]^4 where q1,q2
are shared-slope factored cubics (minimax-fitted, 2.3e-3 max rel err over
|x|<=8.6); beta enters through q1's per-partition scalar operands.
"""

import numpy as np

import concourse.bass as bass
import concourse.mybir as mybir
import concourse.tile as tile
from concourse.bass_utils import run_bass_kernel_spmd
from concourse.vector_clock import ScopedClock

F32 = mybir.dt.float32
F16 = mybir.dt.float16

B, C, D, H, W = 2, 128, 16, 16, 16
N = D * H * W          # 4096 target tokens
NCORES = 8
QSHARDS = NCORES // B  # 4 query shards per batch
NQ = N // QSHARDS      # 1024 query tokens per core
KT = N // 128          # 32 key tiles
QC = NQ // 128         # 8 query chunks of 128
AF = mybir.ActivationFunctionType
OP = mybir.AluOpType

# exp(x) ~ [q2(q1(x))]^4, q_i(x) = (sq(s*x+b)+c)*(s*x+w)  (shared-slope cubics)
S1, B1, C1, W1 = 0.05194748781283326, 0.2171248992897139, 1.4819305023704588, 0.56936452908816459
S2, B2, C2, W2 = 1.0244462795933973, -0.098833807649857053, 0.29508885935180756, 0.19029542731487301

# Custom DVE ops do not compile on this walrus build ("ISA wrong length" in
# codegen even for production ops), so exp runs entirely on the Act engine.
DVE_TILES = frozenset()


# --- walrus sync-wait workarounds (unchanged from baseline) ------------------

def _patched_drain_and_barrier(self, tick_clock, wait_clock):
    # This walrus build caps sync-waits per instruction; the stock TileContext
    # exit drain carries one wait per processor lane (>4 in this kernel).
    # Split the waits into single-wait SP instructions before the drain.
    nc = self.nc
    probe = nc.sync.nop()
    wait_clock.add_sem_waits(probe.ins, ScopedClock({None: tick_clock.global_clock}))
    si = probe.ins.sync_info
    waits = list(si.on_wait) if si and si.on_wait else []
    if si:
        si.on_wait = []
        probe.ins.sync_info = si
    by_name = {h.name: h for h in self.sems.allocated().values()}
    opmap = {"sem-ge-imm": "sem-ge", "sem-eq-imm": "sem-eq"}
    for wv in waits:
        nc.sync.wait_op(by_name[wv.ant_name], wv.wait_value, opmap.get(wv.wait_mode, "sem-ge"))
    nc.sync.drain()
    nc.all_engine_barrier()
    popped = nc._tile_sem_poison_stack.pop()
    assert popped is self._sem_poison
    nc.clear_and_free_semaphores(list(self.sems.allocated().values()))


tile.TileContext._drain_and_barrier = _patched_drain_and_barrier


def _split_excess_waits(nc, cap=1, evsem_cap=2):
    # This walrus build rejects instructions carrying more than ~1 sync wait
    # (Tile targets a newer walrus that packs several). Hoist excess waits
    # onto dedicated InstEventSemaphore instructions just before the
    # over-subscribed instruction, on the same engine stream.
    for fn in nc.m.functions:
        for bb in fn.blocks:
            out = []
            for inst in bb.instructions:
                si = inst.sync_info
                waits = list(si.on_wait) if si and si.on_wait else []
                limit = (
                    evsem_cap
                    if isinstance(inst, (mybir.InstEventSemaphore, mybir.InstDrain))
                    else cap
                )
                if len(waits) > limit:
                    excess, keep = waits[:-limit], waits[-limit:]
                    for i in range(0, len(excess), evsem_cap):
                        ev = mybir.InstEventSemaphore(
                            name=nc.get_next_instruction_name(),
                            engine=inst.engine,
                            ins=[],
                            outs=[],
                            sync_info=mybir.SyncInfo(
                                on_wait=excess[i : i + evsem_cap], on_update=[]
                            ),
                        )
                        nc.register_instruction(ev)
                        out.append(ev)
                    si.on_wait = keep
                    inst.sync_info = si
                out.append(inst)
            bb.instructions[:] = out


# --- kernel ------------------------------------------------------------------

def build_bass():
    nc = bass.Bass("TRN2", target_bir_lowering=False, debug=False)

    srcq = nc.dram_tensor("srcq", [C, NQ], F16, kind="ExternalInput")
    tgt = nc.dram_tensor("tgt", [C, N], F16, kind="ExternalInput")
    mw = nc.dram_tensor("mw", [C, 2, C], F16, kind="ExternalInput")   # M^T | Wvo^T
    bet3 = nc.dram_tensor("bet3", [C, 3, KT], F32, kind="ExternalInput")  # beta|b1'|w1'
    out = nc.dram_tensor("out", [C, QC, C], F16, kind="ExternalOutput")   # [qpart, chunk, co]

    with tile.TileContext(nc) as tc:
        with (
            tc.tile_pool(name="consts", bufs=1) as consts,
            tc.tile_pool(name="big", bufs=1) as big,
            tc.tile_pool(name="ets", bufs=4) as ets,
            tc.tile_pool(name="psum", bufs=2, space="PSUM") as psum,
            tc.tile_pool(name="psum_pv", bufs=1, space="PSUM") as psum_pv,
        ):
            # ---- inputs on two HWDGE rings (SP + ACT); critical tensors first.
            mw_sb = consts.tile([C, 2, C], F16)
            mt_h, wvo_h = mw_sb[:, 0, :], mw_sb[:, 1, :]
            bet3_sb = consts.tile([C, 3, KT], F32)
            beta_sb = bet3_sb[:, 0, :]
            tgt_c = [big.tile([C, 1024], F16, name=f"tgt_c{j}") for j in range(4)]
            srcq_sb = big.tile([C, NQ], F16)

            nc.sync.dma_start(mw_sb[:], mw[:, :, :])
            nc.scalar.dma_start(tgt_c[0][:], tgt[:, 0:1024])
            nc.sync.dma_start(tgt_c[1][:], tgt[:, 1024:2048])
            nc.scalar.dma_start(bet3_sb[:], bet3[:, :, :])
            nc.sync.dma_start(srcq_sb[:], srcq[:, :])
            nc.scalar.dma_start(tgt_c[2][:], tgt[:, 2048:3072])
            nc.sync.dma_start(tgt_c[3][:], tgt[:, 3072:4096])

            warm_src = consts.tile([C, 512], F16)
            nc.gpsimd.memset(warm_src[:], 1.0)
            # V'^T tiles with denominator ones-column: [k-token, kt, c|1].
            # The column write is strided; gpsimd memset crashes the exec unit
            # on strided APs, DVE handles it.
            vta = big.tile([C, KT, C + 1], F16)
            nc.vector.memset(vta[:, :, C : C + 1], 1.0)

            # PE warm-up: dummy matmuls with no DMA deps ramp the HAM clock
            # while the input DMAs are in flight.
            for wi in range(7):
                warm_ps = psum.tile([C, 512], F32, tag="ps_big", bufs=2, name=f"warm_{wi}")
                nc.tensor.matmul(
                    warm_ps[:], warm_src[:, 0:128], warm_src[:], start=True, stop=True,
                )

            # ---- projections ----
            k_c = [big.tile([C, 1024], F16, name=f"k_c{j}") for j in range(4)]

            def emit_kproj(j):
                kp = psum.tile([C, 1024], F32, tag="ps_big", bufs=2, name=f"kp_{j}")
                for h in range(2):
                    nc.tensor.matmul(
                        kp[:, h * 512 : (h + 1) * 512],
                        mt_h,
                        tgt_c[j][:, h * 512 : (h + 1) * 512],
                        start=True,
                        stop=True,
                    )
                nc.vector.tensor_copy(k_c[j][:], kp[:])

            def emit_vproj(g):
                # V'^T[m, c] = (Wvo @ tgt)^T for 8 m-tiles
                vp = psum.tile([C, 8, C], F32, tag="ps_big", bufs=2, name=f"vp_{g}")
                for i in range(8):
                    mt = g * 8 + i
                    nc.tensor.matmul(
                        vp[:, i, :],
                        tgt_c[mt // 8][:, (mt % 8) * C : (mt % 8 + 1) * C],
                        wvo_h,
                        start=True,
                        stop=True,
                    )
                nc.vector.tensor_copy(vta[:, g * 8 : (g + 1) * 8, 0:C], vp[:])

            # ---- attention pipeline ----
            opsum_a = psum_pv.tile([C, 4, 256], F32, name="opsum_a")
            opsum_b = psum_pv.tile([C, 4, 256], F32, name="opsum_b")

            st_tiles = {}
            et_tiles = {}

            def emit_st(kt):
                st = psum.tile([C, NQ], F32, tag="ps_big", bufs=2, name=f"st_{kt}")
                for h in range(2):
                    nc.tensor.matmul(
                        st[:, h * 512 : (h + 1) * 512],
                        k_c[kt // 8][:, (kt % 8) * C : (kt % 8 + 1) * C],
                        srcq_sb[:, h * 512 : (h + 1) * 512],
                        start=True,
                        stop=True,
                    )
                st_tiles[kt] = st

            def emit_exp(kt):
                et = ets.tile([C, NQ], F16, tag="et", name=f"et_{kt}")
                st = st_tiles.pop(kt)
                nc.scalar.activation(
                    out=et[:], in_=st[:], func=AF.Exp,
                    bias=beta_sb[:, kt : kt + 1], scale=1.0,
                )
                et_tiles[kt] = et

            def emit_pv(kt):
                # Chunk pairs share a PSUM bank; a start=True matmul clears the
                # has_written bits for the WHOLE bank, so only the bank's first
                # region may use start. The bank-mate's first write then lands
                # on cleared bits and overwrites-and-sets, which is exactly the
                # accumulation-group opening it needs.
                et = et_tiles.pop(kt)
                for j in range(QC):
                    ops = opsum_a if j < 4 else opsum_b
                    nc.tensor.matmul(
                        ops[:, j % 4, 0 : C + 1],
                        et[:, j * 128 : (j + 1) * 128],
                        vta[:, kt, :],
                        start=(kt == 0 and j % 2 == 0),
                        stop=(kt == KT - 1),
                        skip_group_check=True,
                    )

            # ---- software-pipelined emission ----
            emit_kproj(0)
            emit_vproj(0)
            emit_kproj(1)
            emit_st(0)
            emit_exp(0)
            for kt in range(1, KT):
                if kt == 2:
                    emit_vproj(1)
                elif kt == 5:
                    emit_kproj(2)
                elif kt == 9:
                    emit_vproj(2)
                elif kt == 13:
                    emit_kproj(3)
                elif kt == 17:
                    emit_vproj(3)
                emit_st(kt)
                emit_exp(kt)
                emit_pv(kt - 1)
            emit_pv(KT - 1)

            # ---- epilogue: normalize [q, c] by the ones-column denominators
            recip_sb = big.tile([C, QC], F32)
            o_fin = big.tile([C, QC, C], F16)
            nc.vector.reciprocal(out=recip_sb[:, 0:4], in_=opsum_a[:, :, C])
            nc.vector.reciprocal(out=recip_sb[:, 4:8], in_=opsum_b[:, :, C])
            for j in range(QC):
                ops = opsum_a if j < 4 else opsum_b
                nc.vector.tensor_scalar(
                    out=o_fin[:, j, :], in0=ops[:, j % 4, 0:C],
                    scalar1=recip_sb[:, j : j + 1], scalar2=None, op0=OP.mult,
                )
                if j == 3:
                    nc.sync.dma_start(out[:, 0:4, :], o_fin[:, 0:4, :])
            nc.scalar.dma_start(out[:, 4:8, :], o_fin[:, 4:8, :])

    _split_excess_waits(nc)
    return nc


_NC_CACHE = None


def _get_nc():
    global _NC_CACHE
    if _NC_CACHE is None:
        _NC_CACHE = build_bass()
    return _NC_CACHE


def make_in_maps(source, target, wq, bq, wk, bk, wv, bv, wo, bo):
    source = np.asarray(source, dtype=np.float32).reshape(B, C, N)
    target = np.asarray(target, dtype=np.float32).reshape(B, C, N)
    wq, wk, wv, wo = (np.asarray(x, np.float32) for x in (wq, wk, wv, wo))
    bq, bk, bv, bo = (np.asarray(x, np.float32) for x in (bq, bk, bv, bo))
    scale = np.float32(1.0 / np.sqrt(C))

    M = (wq.T @ wk) * scale                 # [c_src, c_tgt]
    Wvo = wo @ wv                            # [c_out, c_tgt]
    mw_v = np.ascontiguousarray(
        np.stack([M.T, Wvo.T], axis=1).astype(np.float16)
    )                                        # [ct, 2, c]

    src16 = source.astype(np.float16)
    tgt16 = target.astype(np.float16)

    in_maps = []
    bet3_b = []
    for b in range(B):
        beta = ((bq @ (wk @ target[b])) + np.float32(bq @ bk)) * scale  # [N]
        bvec = beta.reshape(KT, 128).T.astype(np.float32)               # [128, KT]
        b1p = np.float32(B1) + np.float32(S1) * bvec
        w1p = np.float32(W1) + np.float32(S1) * bvec
        bet3_b.append(np.ascontiguousarray(
            np.stack([bvec, b1p, w1p], axis=1).astype(np.float32)
        ))                                                              # [128, 3, KT]

    for core in range(NCORES):
        b, qs = divmod(core, QSHARDS)
        in_maps.append({
            "srcq": np.ascontiguousarray(src16[b, :, qs * NQ : (qs + 1) * NQ]),
            "tgt": np.ascontiguousarray(tgt16[b]),
            "mw": mw_v,
            "bet3": bet3_b[b],
        })
    return in_maps


def kernel(source, target, wq, bq, wk, bk, wv, bv, wo, bo):
    nc = _get_nc()
    in_maps = make_in_maps(source, target, wq, bq, wk, bk, wv, bv, wo, bo)
    res = run_bass_kernel_spmd(nc, in_maps, core_ids=list(range(NCORES)))
    bvo = (np.asarray(wo, np.float32) @ np.asarray(bv, np.float32)
           + np.asarray(bo, np.float32))                                # [C]
    full = np.empty((B, C, N), dtype=np.float32)
    for core in range(NCORES):
        b, qs = divmod(core, QSHARDS)
        o = np.asarray(res.results[core]["out"], np.float32)            # [p, j, co]
        full[b, :, qs * NQ : (qs + 1) * NQ] = (
            o.transpose(2, 1, 0).reshape(C, NQ) + bvo[:, None]
        )
    return full.reshape(B, C, D, H, W)


# revision 10
# speedup vs baseline: 1.0813x; 1.0722x over previous
"""CrossAttention3D Trainium2 kernel (restructured).

Full inputs in, full output out. Sharding: data-parallel over batch (2) x
query-token shards (4) = 8 NeuronCores; each core runs 1024 queries against
all 4096 keys.

Host-side algebraic folding removes two of the four projections:
  scores = src^T (Wq^T Wk / sqrt(C)) tgt  -> one folded K-projection, src is
  used directly as the query matrix. The per-key bias term beta_k =
  (bq . Wk tgt_k + bq.bk)/sqrt(C) is folded into exp (per-query terms are
  softmax-invariant and dropped exactly).
  out = wo @ (attn V) + .. -> V' = (wo wv) tgt, so the PV contraction directly
  produces output channels; wo@bv + bo is added on the host.

Attention layout: scores st[k,1024q] per 128-key tile (k on partitions); exp
tiles feed PV as matmul *weights* (128-query chunks) against V'^T tiles
augmented with a ones column [128k, 128c+1], accumulating opsum[q, c+1] in
PSUM across all 32 key tiles — the softmax denominator falls out of the same
matmuls as channel 128, eliminating the entire DVE accumulation chain. The
[q, c] output is transposed on the host.

exp runs split across two engines: Act (hardware Exp, bias=beta) and DVE via
two chained custom-DVE ops computing exp(x+beta) ~ [q2(q1(x))]^4 where q1,q2
are shared-slope factored cubics (minimax-fitted, 2.3e-3 max rel err over
|x|<=8.6); beta enters through q1's per-partition scalar operands.
"""

import numpy as np

import concourse.bass as bass
import concourse.mybir as mybir
import concourse.tile as tile
from concourse.bass_utils import run_bass_kernel_spmd
from concourse.vector_clock import ScopedClock

F32 = mybir.dt.float32
F16 = mybir.dt.float16

B, C, D, H, W = 2, 128, 16, 16, 16
N = D * H * W          # 4096 target tokens
NCORES = 8
QSHARDS = NCORES // B  # 4 query shards per batch
NQ = N // QSHARDS      # 1024 query tokens per core
KT = N // 128          # 32 key tiles
QC = NQ // 128         # 8 query chunks of 128
AF = mybir.ActivationFunctionType
OP = mybir.AluOpType

# exp(x) ~ [q2(q1(x))]^4, q_i(x) = (sq(s*x+b)+c)*(s*x+w)  (shared-slope cubics)
S1, B1, C1, W1 = 0.05194748781283326, 0.2171248992897139, 1.4819305023704588, 0.56936452908816459
S2, B2, C2, W2 = 1.0244462795933973, -0.098833807649857053, 0.29508885935180756, 0.19029542731487301

# Custom DVE ops do not compile on this walrus build ("ISA wrong length" in
# codegen even for production ops), so exp runs entirely on the Act engine.
DVE_TILES = frozenset()


# --- walrus sync-wait workarounds (unchanged from baseline) ------------------

def _patched_drain_and_barrier(self, tick_clock, wait_clock):
    # This walrus build caps sync-waits per instruction; the stock TileContext
    # exit drain carries one wait per processor lane (>4 in this kernel).
    # Split the waits into single-wait SP instructions before the drain.
    nc = self.nc
    probe = nc.sync.nop()
    wait_clock.add_sem_waits(probe.ins, ScopedClock({None: tick_clock.global_clock}))
    si = probe.ins.sync_info
    waits = list(si.on_wait) if si and si.on_wait else []
    if si:
        si.on_wait = []
        probe.ins.sync_info = si
    by_name = {h.name: h for h in self.sems.allocated().values()}
    opmap = {"sem-ge-imm": "sem-ge", "sem-eq-imm": "sem-eq"}
    for wv in waits:
        nc.sync.wait_op(by_name[wv.ant_name], wv.wait_value, opmap.get(wv.wait_mode, "sem-ge"))
    nc.sync.drain()
    nc.all_engine_barrier()
    popped = nc._tile_sem_poison_stack.pop()
    assert popped is self._sem_poison
    nc.clear_and_free_semaphores(list(self.sems.allocated().values()))


tile.TileContext._drain_and_barrier = _patched_drain_and_barrier


def _split_excess_waits(nc, cap=1, evsem_cap=2):
    # This walrus build rejects instructions carrying more than ~1 sync wait
    # (Tile targets a newer walrus that packs several). Hoist excess waits
    # onto dedicated InstEventSemaphore instructions just before the
    # over-subscribed instruction, on the same engine stream.
    for fn in nc.m.functions:
        for bb in fn.blocks:
            out = []
            for inst in bb.instructions:
                si = inst.sync_info
                waits = list(si.on_wait) if si and si.on_wait else []
                limit = (
                    evsem_cap
                    if isinstance(inst, (mybir.InstEventSemaphore, mybir.InstDrain))
                    else cap
                )
                if len(waits) > limit:
                    excess, keep = waits[:-limit], waits[-limit:]
                    for i in range(0, len(excess), evsem_cap):
                        ev = mybir.InstEventSemaphore(
                            name=nc.get_next_instruction_name(),
                            engine=inst.engine,
                            ins=[],
                            outs=[],
                            sync_info=mybir.SyncInfo(
                                on_wait=excess[i : i + evsem_cap], on_update=[]
                            ),
                        )
                        nc.register_instruction(ev)
                        out.append(ev)
                    si.on_wait = keep
                    inst.sync_info = si
                out.append(inst)
            bb.instructions[:] = out


# --- kernel ------------------------------------------------------------------

def build_bass():
    nc = bass.Bass("TRN2", target_bir_lowering=False, debug=False)

    srcq = nc.dram_tensor("srcq", [C, NQ], F16, kind="ExternalInput")
    tgt = nc.dram_tensor("tgt", [C, N], F16, kind="ExternalInput")
    mw = nc.dram_tensor("mw", [C, 2, C], F16, kind="ExternalInput")   # M^T | Wvo^T
    bet3 = nc.dram_tensor("bet3", [C, 3, KT], F32, kind="ExternalInput")  # beta|b1'|w1'
    out = nc.dram_tensor("out", [C, QC, C], F16, kind="ExternalOutput")   # [qpart, chunk, co]

    # opsum chunk j -> (psum tile, region index). Three 129-wide fp32 regions
    # at 170-float stride fit one 2KB bank; start=True only on region 0 (the
    # bank-wide has_written clear opens the bank-mates' groups too).
    CHUNK_MAP = [(0, 0), (0, 1), (0, 2), (1, 0), (1, 1), (1, 2), (2, 0), (2, 1)]

    with tile.TileContext(nc) as tc:
        with (
            tc.tile_pool(name="consts", bufs=1) as consts,
            tc.tile_pool(name="big", bufs=1) as big,
            tc.tile_pool(name="ets", bufs=4) as ets,
            tc.tile_pool(name="psum", bufs=2, space="PSUM") as psum,
            tc.tile_pool(name="psum_pv", bufs=1, space="PSUM") as psum_pv,
            tc.tile_pool(name="psum_kv", bufs=1, space="PSUM") as psum_kv,
        ):
            warm_src = consts.tile([C, 512], F16)
            nc.gpsimd.memset(warm_src[:], 1.0)

            # ---- inputs on two HWDGE rings (SP + ACT); critical tensors first.
            mw_sb = consts.tile([C, 2, C], F16)
            mt_h, wvo_h = mw_sb[:, 0, :], mw_sb[:, 1, :]
            bet3_sb = consts.tile([C, 3, KT], F32)
            beta_sb = bet3_sb[:, 0, :]
            tgt_c = [big.tile([C, 1024], F16, name=f"tgt_c{j}") for j in range(4)]
            srcq_sb = big.tile([C, NQ], F16)

            nc.scalar.dma_start(tgt_c[0][:], tgt[:, 0:1024])
            nc.sync.dma_start(mw_sb[:], mw[:, :, :])
            nc.sync.dma_start(srcq_sb[:], srcq[:, :])
            nc.scalar.dma_start(bet3_sb[:], bet3[:, :, :])
            nc.sync.dma_start(tgt_c[1][:], tgt[:, 1024:2048])
            nc.scalar.dma_start(tgt_c[2][:], tgt[:, 2048:3072])
            nc.sync.dma_start(tgt_c[3][:], tgt[:, 3072:4096])

            # V'^T tiles with denominator ones-column: [k-token, kt, c|1].
            # The column write is strided; gpsimd memset crashes the exec unit
            # on strided APs, DVE handles it.
            vta = big.tile([C, KT, C + 1], F16)
            nc.vector.memset(vta[:, :, C : C + 1], 1.0)

            # PE warm-up: dummy matmuls with no DMA deps ramp the HAM clock
            # while the input DMAs are in flight.
            for wi in range(6):
                warm_ps = psum.tile([C, 512], F32, tag="ps_big", bufs=2, name=f"warm_{wi}")
                nc.tensor.matmul(
                    warm_ps[:], warm_src[:, 0:128], warm_src[:], start=True, stop=True,
                )

            # ---- projections: all pieces share ONE spare PSUM bank, so they
            # never contend with the score-tile ring; each piece is a matmul
            # plus a DVE convert, self-serialized through the bank.
            k_c = [big.tile([C, 1024], F16, name=f"k_c{j}") for j in range(4)]

            def emit_kv(piece):
                kind, idx = piece[0], int(piece[1:])
                if kind == "k":
                    j, h = divmod(idx, 2)
                    kvp = psum_kv.tile([C, 512], F32, tag="kv", bufs=1, name=f"kp{idx}")
                    nc.tensor.matmul(
                        kvp[:], mt_h, tgt_c[j][:, h * 512 : (h + 1) * 512],
                        start=True, stop=True,
                    )
                    nc.vector.tensor_copy(k_c[j][:, h * 512 : (h + 1) * 512], kvp[:])
                else:
                    g = idx
                    kvp = psum_kv.tile([C, 4, C], F32, tag="kv", bufs=1, name=f"vp{idx}")
                    for i in range(4):
                        mt = g * 4 + i
                        nc.tensor.matmul(
                            kvp[:, i, :],
                            tgt_c[mt // 8][:, (mt % 8) * C : (mt % 8 + 1) * C],
                            wvo_h, start=True, stop=True,
                        )
                    nc.vector.tensor_copy(vta[:, g * 4 : (g + 1) * 4, 0:C], kvp[:])

            # ---- attention pipeline ----
            opsum = [
                psum_pv.tile([C, 3, 170], F32, name="opsum_a"),
                psum_pv.tile([C, 3, 170], F32, name="opsum_b"),
                psum_pv.tile([C, 2, 170], F32, name="opsum_c"),
            ]

            st_tiles = {}
            et_tiles = {}

            def emit_st(kt):
                st = psum.tile([C, NQ], F32, tag="ps_big", bufs=2, name=f"st_{kt}")
                for h in range(2):
                    nc.tensor.matmul(
                        st[:, h * 512 : (h + 1) * 512],
                        k_c[kt // 8][:, (kt % 8) * C : (kt % 8 + 1) * C],
                        srcq_sb[:, h * 512 : (h + 1) * 512],
                        start=True, stop=True,
                    )
                st_tiles[kt] = st

            def emit_exp(kt):
                et = ets.tile([C, NQ], F16, tag="et", name=f"et_{kt}")
                st = st_tiles.pop(kt)
                nc.scalar.activation(
                    out=et[:], in_=st[:], func=AF.Exp,
                    bias=beta_sb[:, kt : kt + 1], scale=1.0,
                )
                et_tiles[kt] = et

            def emit_pv(kt):
                et = et_tiles.pop(kt)
                for j in range(QC):
                    t, idx = CHUNK_MAP[j]
                    nc.tensor.matmul(
                        opsum[t][:, idx, 0 : C + 1],
                        et[:, j * 128 : (j + 1) * 128],
                        vta[:, kt, :],
                        start=(kt == 0 and idx == 0),
                        stop=(kt == KT - 1),
                        skip_group_check=True,
                    )

            # ---- software-pipelined emission; kv pieces in deadline order.
            kv_order = [
                "k0", "v0", "k1", "v1", "k2", "v2", "k3", "v3",
                "k4", "v4", "k5", "v5", "k6", "v6", "k7", "v7",
            ]
            emit_kv(kv_order[0])
            emit_kv(kv_order[1])
            emit_kv(kv_order[2])
            emit_kv(kv_order[3])
            emit_st(0)
            emit_exp(0)
            nkv = 4
            for kt in range(1, KT):
                if nkv < len(kv_order):
                    emit_kv(kv_order[nkv])
                    nkv += 1
                emit_st(kt)
                emit_exp(kt)
                emit_pv(kt - 1)
            emit_pv(KT - 1)

            # ---- epilogue: normalize [q, c] by the ones-column denominators;
            # norm ops split across Act (idle now) and DVE, DMA per half.
            recip_sb = big.tile([C, QC], F32)
            o_fin = big.tile([C, QC, C], F16)
            nc.vector.reciprocal(out=recip_sb[:, 0:3], in_=opsum[0][:, :, C])
            nc.vector.reciprocal(out=recip_sb[:, 3:6], in_=opsum[1][:, :, C])
            nc.vector.reciprocal(out=recip_sb[:, 6:8], in_=opsum[2][:, :, C])
            for j in range(QC):
                t, idx = CHUNK_MAP[j]
                if j % 2 == 0:
                    nc.scalar.activation(
                        out=o_fin[:, j, :], in_=opsum[t][:, idx, 0:C],
                        func=AF.Identity, scale=recip_sb[:, j : j + 1],
                    )
                else:
                    nc.vector.tensor_scalar(
                        out=o_fin[:, j, :], in0=opsum[t][:, idx, 0:C],
                        scalar1=recip_sb[:, j : j + 1], scalar2=None, op0=OP.mult,
                    )
                if j == 3:
                    nc.sync.dma_start(out[:, 0:4, :], o_fin[:, 0:4, :])
            nc.scalar.dma_start(out[:, 4:8, :], o_fin[:, 4:8, :])

    _split_excess_waits(nc)
    return nc


_NC_CACHE = None


def _get_nc():
    global _NC_CACHE
    if _NC_CACHE is None:
        _NC_CACHE = build_bass()
    return _NC_CACHE


def make_in_maps(source, target, wq, bq, wk, bk, wv, bv, wo, bo):
    source = np.asarray(source, dtype=np.float32).reshape(B, C, N)
    target = np.asarray(target, dtype=np.float32).reshape(B, C, N)
    wq, wk, wv, wo = (np.asarray(x, np.float32) for x in (wq, wk, wv, wo))
    bq, bk, bv, bo = (np.asarray(x, np.float32) for x in (bq, bk, bv, bo))
    scale = np.float32(1.0 / np.sqrt(C))

    M = (wq.T @ wk) * scale                 # [c_src, c_tgt]
    Wvo = wo @ wv                            # [c_out, c_tgt]
    mw_v = np.ascontiguousarray(
        np.stack([M.T, Wvo.T], axis=1).astype(np.float16)
    )                                        # [ct, 2, c]

    src16 = source.astype(np.float16)
    tgt16 = target.astype(np.float16)

    in_maps = []
    bet3_b = []
    for b in range(B):
        beta = ((bq @ (wk @ target[b])) + np.float32(bq @ bk)) * scale  # [N]
        bvec = beta.reshape(KT, 128).T.astype(np.float32)               # [128, KT]
        b1p = np.float32(B1) + np.float32(S1) * bvec
        w1p = np.float32(W1) + np.float32(S1) * bvec
        bet3_b.append(np.ascontiguousarray(
            np.stack([bvec, b1p, w1p], axis=1).astype(np.float32)
        ))                                                              # [128, 3, KT]

    for core in range(NCORES):
        b, qs = divmod(core, QSHARDS)
        in_maps.append({
            "srcq": np.ascontiguousarray(src16[b, :, qs * NQ : (qs + 1) * NQ]),
            "tgt": np.ascontiguousarray(tgt16[b]),
            "mw": mw_v,
            "bet3": bet3_b[b],
        })
    return in_maps


def kernel(source, target, wq, bq, wk, bk, wv, bv, wo, bo):
    nc = _get_nc()
    in_maps = make_in_maps(source, target, wq, bq, wk, bk, wv, bv, wo, bo)
    res = run_bass_kernel_spmd(nc, in_maps, core_ids=list(range(NCORES)))
    bvo = (np.asarray(wo, np.float32) @ np.asarray(bv, np.float32)
           + np.asarray(bo, np.float32))                                # [C]
    full = np.empty((B, C, N), dtype=np.float32)
    for core in range(NCORES):
        b, qs = divmod(core, QSHARDS)
        o = np.asarray(res.results[core]["out"], np.float32)            # [p, j, co]
        full[b, :, qs * NQ : (qs + 1) * NQ] = (
            o.transpose(2, 1, 0).reshape(C, NQ) + bvo[:, None]
        )
    return full.reshape(B, C, D, H, W)


# revision 11
# speedup vs baseline: 1.1032x; 1.0202x over previous
"""CrossAttention3D Trainium2 kernel (restructured).

Full inputs in, full output out. Sharding: data-parallel over batch (2) x
query-token shards (4) = 8 NeuronCores; each core runs 1024 queries against
all 4096 keys.

Host-side algebraic folding removes two of the four projections:
  scores = src^T (Wq^T Wk / sqrt(C)) tgt  -> one folded K-projection, src is
  used directly as the query matrix. The per-key bias term beta_k =
  (bq . Wk tgt_k + bq.bk)/sqrt(C) is folded into exp (per-query terms are
  softmax-invariant and dropped exactly).
  out = wo @ (attn V) + .. -> V' = (wo wv) tgt, so the PV contraction directly
  produces output channels; wo@bv + bo is added on the host.

Attention layout: scores st[k,1024q] per 128-key tile (k on partitions); exp
tiles feed PV as matmul *weights* (128-query chunks) against V'^T tiles
augmented with a ones column [128k, 128c+1], accumulating opsum[q, c+1] in
PSUM across all 32 key tiles — the softmax denominator falls out of the same
matmuls as channel 128, eliminating the entire DVE accumulation chain. The
[q, c] output is transposed on the host.

exp runs split across two engines: Act (hardware Exp, bias=beta) and DVE via
two chained custom-DVE ops computing exp(x+beta) ~ [q2(q1(x))]^4 where q1,q2
are shared-slope factored cubics (minimax-fitted, 2.3e-3 max rel err over
|x|<=8.6); beta enters through q1's per-partition scalar operands.
"""

import numpy as np

import concourse.bass as bass
import concourse.mybir as mybir
import concourse.tile as tile
from concourse.bass_utils import run_bass_kernel_spmd
from concourse.vector_clock import ScopedClock

F32 = mybir.dt.float32
F16 = mybir.dt.float16

B, C, D, H, W = 2, 128, 16, 16, 16
N = D * H * W          # 4096 target tokens
NCORES = 8
QSHARDS = NCORES // B  # 4 query shards per batch
NQ = N // QSHARDS      # 1024 query tokens per core
KT = N // 128          # 32 key tiles
QC = NQ // 128         # 8 query chunks of 128
AF = mybir.ActivationFunctionType
OP = mybir.AluOpType

# exp(x) ~ [q2(q1(x))]^4, q_i(x) = (sq(s*x+b)+c)*(s*x+w)  (shared-slope cubics)
S1, B1, C1, W1 = 0.05194748781283326, 0.2171248992897139, 1.4819305023704588, 0.56936452908816459
S2, B2, C2, W2 = 1.0244462795933973, -0.098833807649857053, 0.29508885935180756, 0.19029542731487301

# Custom DVE ops do not compile on this walrus build ("ISA wrong length" in
# codegen even for production ops), so exp runs entirely on the Act engine.
DVE_TILES = frozenset()


# --- walrus sync-wait workarounds (unchanged from baseline) ------------------

def _patched_drain_and_barrier(self, tick_clock, wait_clock):
    # This walrus build caps sync-waits per instruction; the stock TileContext
    # exit drain carries one wait per processor lane (>4 in this kernel).
    # Split the waits into single-wait SP instructions before the drain.
    nc = self.nc
    probe = nc.sync.nop()
    wait_clock.add_sem_waits(probe.ins, ScopedClock({None: tick_clock.global_clock}))
    si = probe.ins.sync_info
    waits = list(si.on_wait) if si and si.on_wait else []
    if si:
        si.on_wait = []
        probe.ins.sync_info = si
    by_name = {h.name: h for h in self.sems.allocated().values()}
    opmap = {"sem-ge-imm": "sem-ge", "sem-eq-imm": "sem-eq"}
    for wv in waits:
        nc.sync.wait_op(by_name[wv.ant_name], wv.wait_value, opmap.get(wv.wait_mode, "sem-ge"))
    nc.sync.drain()
    nc.all_engine_barrier()
    popped = nc._tile_sem_poison_stack.pop()
    assert popped is self._sem_poison
    nc.clear_and_free_semaphores(list(self.sems.allocated().values()))


tile.TileContext._drain_and_barrier = _patched_drain_and_barrier


def _split_excess_waits(nc, cap=1, evsem_cap=2):
    # This walrus build rejects instructions carrying more than ~1 sync wait
    # (Tile targets a newer walrus that packs several). Hoist excess waits
    # onto dedicated InstEventSemaphore instructions just before the
    # over-subscribed instruction, on the same engine stream.
    for fn in nc.m.functions:
        for bb in fn.blocks:
            out = []
            for inst in bb.instructions:
                si = inst.sync_info
                waits = list(si.on_wait) if si and si.on_wait else []
                limit = (
                    evsem_cap
                    if isinstance(inst, (mybir.InstEventSemaphore, mybir.InstDrain))
                    else cap
                )
                if len(waits) > limit:
                    excess, keep = waits[:-limit], waits[-limit:]
                    for i in range(0, len(excess), evsem_cap):
                        ev = mybir.InstEventSemaphore(
                            name=nc.get_next_instruction_name(),
                            engine=inst.engine,
                            ins=[],
                            outs=[],
                            sync_info=mybir.SyncInfo(
                                on_wait=excess[i : i + evsem_cap], on_update=[]
                            ),
                        )
                        nc.register_instruction(ev)
                        out.append(ev)
                    si.on_wait = keep
                    inst.sync_info = si
                out.append(inst)
            bb.instructions[:] = out


# --- kernel ------------------------------------------------------------------

def build_bass():
    nc = bass.Bass("TRN2", target_bir_lowering=False, debug=False)

    srcq = nc.dram_tensor("srcq", [C, NQ], F16, kind="ExternalInput")
    tgt = nc.dram_tensor("tgt", [C, N], F16, kind="ExternalInput")
    mw = nc.dram_tensor("mw", [C, 2, C], F16, kind="ExternalInput")   # M^T | Wvo^T
    bet3 = nc.dram_tensor("bet3", [C, 3, KT], F32, kind="ExternalInput")  # beta|b1'|w1'
    out = nc.dram_tensor("out", [C, QC, C], F16, kind="ExternalOutput")   # [qpart, chunk, co]

    # opsum chunk j -> (psum tile, region index). Three 129-wide fp32 regions
    # at 170-float stride fit one 2KB bank; start=True only on region 0 (the
    # bank-wide has_written clear opens the bank-mates' groups too).
    CHUNK_MAP = [(0, 0), (0, 1), (0, 2), (1, 0), (1, 1), (1, 2), (2, 0), (2, 1)]

    with tile.TileContext(nc) as tc:
        with (
            tc.tile_pool(name="consts", bufs=1) as consts,
            tc.tile_pool(name="big", bufs=1) as big,
            tc.tile_pool(name="ets", bufs=4) as ets,
            tc.tile_pool(name="psum", bufs=2, space="PSUM") as psum,
            tc.tile_pool(name="psum_pv", bufs=1, space="PSUM") as psum_pv,
            tc.tile_pool(name="psum_kv", bufs=1, space="PSUM") as psum_kv,
        ):
            warm_src = consts.tile([C, 512], F16)
            nc.gpsimd.memset(warm_src[:], 1.0)

            # ---- inputs on two HWDGE rings (SP + ACT); critical tensors first.
            mw_sb = consts.tile([C, 2, C], F16)
            mt_h, wvo_h = mw_sb[:, 0, :], mw_sb[:, 1, :]
            bet3_sb = consts.tile([C, 3, KT], F32)
            beta_sb = bet3_sb[:, 0, :]
            tgt_c = [big.tile([C, 1024], F16, name=f"tgt_c{j}") for j in range(4)]
            srcq_sb = big.tile([C, NQ], F16)

            nc.scalar.dma_start(tgt_c[0][:], tgt[:, 0:1024])
            nc.sync.dma_start(mw_sb[:], mw[:, :, :])
            nc.sync.dma_start(srcq_sb[:], srcq[:, :])
            nc.scalar.dma_start(bet3_sb[:], bet3[:, :, :])
            nc.sync.dma_start(tgt_c[1][:], tgt[:, 1024:2048])
            nc.scalar.dma_start(tgt_c[2][:], tgt[:, 2048:3072])
            nc.sync.dma_start(tgt_c[3][:], tgt[:, 3072:4096])

            # V'^T tiles with denominator ones-column: [k-token, kt, c|1].
            # The column write is strided; gpsimd memset crashes the exec unit
            # on strided APs, DVE handles it.
            vta = big.tile([C, KT, C + 1], F16)
            nc.vector.memset(vta[:, :, C : C + 1], 1.0)
            zero_t = consts.tile([C, 1], F32)
            nc.vector.memset(zero_t[:], 0.0)

            # PE warm-up: dummy matmuls with no DMA deps ramp the HAM clock
            # while the input DMAs are in flight.
            for wi in range(6):
                warm_ps = psum.tile([C, 512], F32, tag="ps_big", bufs=2, name=f"warm_{wi}")
                nc.tensor.matmul(
                    warm_ps[:], warm_src[:, 0:128], warm_src[:], start=True, stop=True,
                )

            # ---- projections: all pieces share ONE spare PSUM bank, so they
            # never contend with the score-tile ring; each piece is a matmul
            # plus a DVE convert, self-serialized through the bank.
            k_c = [big.tile([C, 1024], F16, name=f"k_c{j}") for j in range(4)]

            def emit_kv(piece):
                kind, idx = piece[0], int(piece[1:])
                if kind == "k":
                    j, h = divmod(idx, 2)
                    kvp = psum_kv.tile([C, 512], F32, tag="kv", bufs=1, name=f"kp{idx}")
                    nc.tensor.matmul(
                        kvp[:], mt_h, tgt_c[j][:, h * 512 : (h + 1) * 512],
                        start=True, stop=True,
                    )
                    nc.vector.tensor_copy(k_c[j][:, h * 512 : (h + 1) * 512], kvp[:])
                else:
                    g = idx
                    kvp = psum_kv.tile([C, 2, C], F32, tag="kv", bufs=1, name=f"vp{idx}")
                    for i in range(2):
                        mt = g * 2 + i
                        nc.tensor.matmul(
                            kvp[:, i, :],
                            tgt_c[mt // 8][:, (mt % 8) * C : (mt % 8 + 1) * C],
                            wvo_h, start=True, stop=True,
                        )
                    nc.vector.tensor_copy(vta[:, g * 2 : (g + 1) * 2, 0:C], kvp[:])

            # ---- attention pipeline ----
            opsum = [
                psum_pv.tile([C, 3, 170], F32, name="opsum_a"),
                psum_pv.tile([C, 3, 170], F32, name="opsum_b"),
                psum_pv.tile([C, 2, 170], F32, name="opsum_c"),
            ]

            st_tiles = {}
            et_tiles = {}

            def emit_st(kt):
                st = psum.tile([C, NQ], F32, tag="ps_big", bufs=2, name=f"st_{kt}")
                for h in range(2):
                    nc.tensor.matmul(
                        st[:, h * 512 : (h + 1) * 512],
                        k_c[kt // 8][:, (kt % 8) * C : (kt % 8 + 1) * C],
                        srcq_sb[:, h * 512 : (h + 1) * 512],
                        start=True, stop=True,
                    )
                st_tiles[kt] = st

            def emit_exp(kt):
                et = ets.tile([C, NQ], F16, tag="et", name=f"et_{kt}")
                st = st_tiles.pop(kt)
                nc.scalar.activation(
                    out=et[:], in_=st[:], func=AF.Exp,
                    bias=beta_sb[:, kt : kt + 1], scale=1.0,
                )
                et_tiles[kt] = et

            def emit_pv(kt):
                et = et_tiles.pop(kt)
                for j in range(QC):
                    t, idx = CHUNK_MAP[j]
                    nc.tensor.matmul(
                        opsum[t][:, idx, 0 : C + 1],
                        et[:, j * 128 : (j + 1) * 128],
                        vta[:, kt, :],
                        start=(kt == 0 and idx == 0),
                        stop=(kt == KT - 1),
                        skip_group_check=True,
                    )

            # ---- software-pipelined emission; kv pieces in deadline order.
            kv_order = [
                "v1", "k1", "v2", "v3", "k2", "v4", "v5", "k3", "v6", "v7",
                "k4", "v8", "v9", "k5", "v10", "v11", "k6", "v12", "v13",
                "k7", "v14", "v15",
            ]
            emit_kv("k0")
            emit_kv("v0")
            emit_st(0)
            emit_exp(0)
            nkv = 0
            for kt in range(1, KT):
                emit_st(kt)
                emit_exp(kt)
                if nkv < len(kv_order):
                    emit_kv(kv_order[nkv])
                    nkv += 1
                emit_pv(kt - 1)
            emit_pv(KT - 1)

            # ---- epilogue: normalize [q, c] by the ones-column denominators;
            # norm ops split across Act (idle now) and DVE, DMA per half.
            recip_sb = big.tile([C, QC], F32)
            o_fin = big.tile([C, QC, C], F16)
            nc.vector.reciprocal(out=recip_sb[:, 0:3], in_=opsum[0][:, :, C])
            nc.vector.reciprocal(out=recip_sb[:, 3:6], in_=opsum[1][:, :, C])
            nc.vector.reciprocal(out=recip_sb[:, 6:8], in_=opsum[2][:, :, C])
            for j in range(QC):
                t, idx = CHUNK_MAP[j]
                if j % 2 == 0:
                    nc.scalar.activation(
                        out=o_fin[:, j, :], in_=opsum[t][:, idx, 0:C],
                        func=AF.Identity, bias=zero_t[:],
                        scale=recip_sb[:, j : j + 1],
                    )
                else:
                    nc.vector.tensor_scalar(
                        out=o_fin[:, j, :], in0=opsum[t][:, idx, 0:C],
                        scalar1=recip_sb[:, j : j + 1], scalar2=None, op0=OP.mult,
                    )
                if j % 2 == 1:
                    ring = nc.sync if (j // 2) % 2 == 0 else nc.scalar
                    ring.dma_start(out[:, j - 1 : j + 1, :], o_fin[:, j - 1 : j + 1, :])

    _split_excess_waits(nc)
    return nc


_NC_CACHE = None


def _get_nc():
    global _NC_CACHE
    if _NC_CACHE is None:
        _NC_CACHE = build_bass()
    return _NC_CACHE


def make_in_maps(source, target, wq, bq, wk, bk, wv, bv, wo, bo):
    source = np.asarray(source, dtype=np.float32).reshape(B, C, N)
    target = np.asarray(target, dtype=np.float32).reshape(B, C, N)
    wq, wk, wv, wo = (np.asarray(x, np.float32) for x in (wq, wk, wv, wo))
    bq, bk, bv, bo = (np.asarray(x, np.float32) for x in (bq, bk, bv, bo))
    scale = np.float32(1.0 / np.sqrt(C))

    M = (wq.T @ wk) * scale                 # [c_src, c_tgt]
    Wvo = wo @ wv                            # [c_out, c_tgt]
    mw_v = np.ascontiguousarray(
        np.stack([M.T, Wvo.T], axis=1).astype(np.float16)
    )                                        # [ct, 2, c]

    src16 = source.astype(np.float16)
    tgt16 = target.astype(np.float16)

    in_maps = []
    bet3_b = []
    for b in range(B):
        beta = ((bq @ (wk @ target[b])) + np.float32(bq @ bk)) * scale  # [N]
        bvec = beta.reshape(KT, 128).T.astype(np.float32)               # [128, KT]
        b1p = np.float32(B1) + np.float32(S1) * bvec
        w1p = np.float32(W1) + np.float32(S1) * bvec
        bet3_b.append(np.ascontiguousarray(
            np.stack([bvec, b1p, w1p], axis=1).astype(np.float32)
        ))                                                              # [128, 3, KT]

    for core in range(NCORES):
        b, qs = divmod(core, QSHARDS)
        in_maps.append({
            "srcq": np.ascontiguousarray(src16[b, :, qs * NQ : (qs + 1) * NQ]),
            "tgt": np.ascontiguousarray(tgt16[b]),
            "mw": mw_v,
            "bet3": bet3_b[b],
        })
    return in_maps


def kernel(source, target, wq, bq, wk, bk, wv, bv, wo, bo):
    nc = _get_nc()
    in_maps = make_in_maps(source, target, wq, bq, wk, bk, wv, bv, wo, bo)
    res = run_bass_kernel_spmd(nc, in_maps, core_ids=list(range(NCORES)))
    bvo = (np.asarray(wo, np.float32) @ np.asarray(bv, np.float32)
           + np.asarray(bo, np.float32))                                # [C]
    full = np.empty((B, C, N), dtype=np.float32)
    for core in range(NCORES):
        b, qs = divmod(core, QSHARDS)
        o = np.asarray(res.results[core]["out"], np.float32)            # [p, j, co]
        full[b, :, qs * NQ : (qs + 1) * NQ] = (
            o.transpose(2, 1, 0).reshape(C, NQ) + bvo[:, None]
        )
    return full.reshape(B, C, D, H, W)


# revision 13
# speedup vs baseline: 1.1601x; 1.0515x over previous
"""CrossAttention3D Trainium2 kernel (restructured).

Full inputs in, full output out. Sharding: data-parallel over batch (2) x
query-token shards (4) = 8 NeuronCores; each core runs 1024 queries against
all 4096 keys.

Host-side algebraic folding removes two of the four projections:
  scores = src^T (Wq^T Wk / sqrt(C)) tgt  -> one folded K-projection, src is
  used directly as the query matrix. The per-key bias term beta_k =
  (bq . Wk tgt_k + bq.bk)/sqrt(C) is folded into exp (per-query terms are
  softmax-invariant and dropped exactly).
  out = wo @ (attn V) + .. -> V' = (wo wv) tgt, so the PV contraction directly
  produces output channels; wo@bv + bo is added on the host.

Attention layout: scores st[k,1024q] per 128-key tile (k on partitions); exp
tiles feed PV as matmul *weights* (128-query chunks) against V'^T tiles
augmented with a ones column [128k, 128c+1], accumulating opsum[q, c+1] in
PSUM across all 32 key tiles — the softmax denominator falls out of the same
matmuls as channel 128, eliminating the entire DVE accumulation chain. The
[q, c] output is transposed on the host.

exp runs split across two engines: Act (hardware Exp, bias=beta) and DVE via
two chained custom-DVE ops computing exp(x+beta) ~ [q2(q1(x))]^4 where q1,q2
are shared-slope factored cubics (minimax-fitted, 2.3e-3 max rel err over
|x|<=8.6); beta enters through q1's per-partition scalar operands.
"""

import numpy as np

import concourse.bass as bass
import concourse.mybir as mybir
import concourse.tile as tile
from concourse.bass_utils import run_bass_kernel_spmd
from concourse.vector_clock import ScopedClock

F32 = mybir.dt.float32
F16 = mybir.dt.float16

B, C, D, H, W = 2, 128, 16, 16, 16
N = D * H * W          # 4096 target tokens
NCORES = 8
QSHARDS = NCORES // B  # 4 query shards per batch
NQ = N // QSHARDS      # 1024 query tokens per core
KT = N // 128          # 32 key tiles
QC = NQ // 128         # 8 query chunks of 128
AF = mybir.ActivationFunctionType
OP = mybir.AluOpType

# exp(x) ~ [q2(q1(x))]^4, q_i(x) = (sq(s*x+b)+c)*(s*x+w)  (shared-slope cubics)
S1, B1, C1, W1 = 0.05194748781283326, 0.2171248992897139, 1.4819305023704588, 0.56936452908816459
S2, B2, C2, W2 = 1.0244462795933973, -0.098833807649857053, 0.29508885935180756, 0.19029542731487301

# Custom DVE ops do not compile on this walrus build ("ISA wrong length" in
# codegen even for production ops), so exp runs entirely on the Act engine.
DVE_TILES = frozenset()


# --- walrus sync-wait workarounds (unchanged from baseline) ------------------

def _patched_drain_and_barrier(self, tick_clock, wait_clock):
    # This walrus build caps sync-waits per instruction; the stock TileContext
    # exit drain carries one wait per processor lane (>4 in this kernel).
    # Split the waits into single-wait SP instructions before the drain.
    nc = self.nc
    probe = nc.sync.nop()
    wait_clock.add_sem_waits(probe.ins, ScopedClock({None: tick_clock.global_clock}))
    si = probe.ins.sync_info
    waits = list(si.on_wait) if si and si.on_wait else []
    if si:
        si.on_wait = []
        probe.ins.sync_info = si
    by_name = {h.name: h for h in self.sems.allocated().values()}
    opmap = {"sem-ge-imm": "sem-ge", "sem-eq-imm": "sem-eq"}
    for wv in waits:
        nc.sync.wait_op(by_name[wv.ant_name], wv.wait_value, opmap.get(wv.wait_mode, "sem-ge"))
    nc.sync.drain()
    nc.all_engine_barrier()
    popped = nc._tile_sem_poison_stack.pop()
    assert popped is self._sem_poison
    nc.clear_and_free_semaphores(list(self.sems.allocated().values()))


tile.TileContext._drain_and_barrier = _patched_drain_and_barrier


def _split_excess_waits(nc, cap=1, evsem_cap=2):
    # This walrus build rejects instructions carrying more than ~1 sync wait
    # (Tile targets a newer walrus that packs several). Hoist excess waits
    # onto dedicated InstEventSemaphore instructions just before the
    # over-subscribed instruction, on the same engine stream.
    for fn in nc.m.functions:
        for bb in fn.blocks:
            out = []
            for inst in bb.instructions:
                si = inst.sync_info
                waits = list(si.on_wait) if si and si.on_wait else []
                limit = (
                    evsem_cap
                    if isinstance(inst, (mybir.InstEventSemaphore, mybir.InstDrain))
                    else cap
                )
                if len(waits) > limit:
                    excess, keep = waits[:-limit], waits[-limit:]
                    for i in range(0, len(excess), evsem_cap):
                        ev = mybir.InstEventSemaphore(
                            name=nc.get_next_instruction_name(),
                            engine=inst.engine,
                            ins=[],
                            outs=[],
                            sync_info=mybir.SyncInfo(
                                on_wait=excess[i : i + evsem_cap], on_update=[]
                            ),
                        )
                        nc.register_instruction(ev)
                        out.append(ev)
                    si.on_wait = keep
                    inst.sync_info = si
                out.append(inst)
            bb.instructions[:] = out


# --- kernel ------------------------------------------------------------------

def build_bass():
    nc = bass.Bass("TRN2", target_bir_lowering=False, debug=False)

    srcq = nc.dram_tensor("srcq", [C, NQ], F16, kind="ExternalInput")
    tgt = nc.dram_tensor("tgt", [C, N], F16, kind="ExternalInput")
    mw = nc.dram_tensor("mw", [C, 2, C], F16, kind="ExternalInput")   # M^T | Wvo^T
    bet3 = nc.dram_tensor("bet3", [C, 3, KT], F32, kind="ExternalInput")  # beta|b1'|w1'
    out = nc.dram_tensor("out", [C, QC, C], F16, kind="ExternalOutput")   # [qpart, chunk, co]

    # opsum chunk j -> (psum tile, region index). Three 129-wide fp32 regions
    # at 170-float stride fit one 2KB bank; start=True only on region 0 (the
    # bank-wide has_written clear opens the bank-mates' groups too).
    CHUNK_MAP = [(0, 0), (0, 1), (0, 2), (1, 0), (1, 1), (1, 2), (2, 0), (2, 1)]

    with tile.TileContext(nc) as tc:
        with (
            tc.tile_pool(name="consts", bufs=1) as consts,
            tc.tile_pool(name="big", bufs=1) as big,
            tc.tile_pool(name="ets", bufs=4) as ets,
            tc.tile_pool(name="psum", bufs=2, space="PSUM") as psum,
            tc.tile_pool(name="psum_pv", bufs=1, space="PSUM") as psum_pv,
            tc.tile_pool(name="psum_kv", bufs=1, space="PSUM") as psum_kv,
        ):
            warm_src = consts.tile([C, 512], F16)
            nc.vector.memset(warm_src[:], 1.0)

            # ---- inputs on two HWDGE rings (SP + ACT); critical tensors first.
            # tgt0 is split so the first K-projection piece can start early.
            mw_sb = consts.tile([C, 2, C], F16)
            mt_h, wvo_h = mw_sb[:, 0, :], mw_sb[:, 1, :]
            bet3_sb = consts.tile([C, 3, KT], F32)
            beta_sb = bet3_sb[:, 0, :]
            tgt_c = [big.tile([C, 1024], F16, name=f"tgt_c{j}") for j in range(4)]
            srcq_sb = big.tile([C, NQ], F16)

            nc.sync.dma_start(tgt_c[0][:, 0:512], tgt[:, 0:512])
            nc.scalar.dma_start(mw_sb[:], mw[:, :, :])
            nc.sync.dma_start(srcq_sb[:], srcq[:, :])
            nc.scalar.dma_start(bet3_sb[:], bet3[:, :, :])
            nc.sync.dma_start(tgt_c[0][:, 512:1024], tgt[:, 512:1024])
            nc.scalar.dma_start(tgt_c[1][:], tgt[:, 1024:2048])
            nc.sync.dma_start(tgt_c[2][:], tgt[:, 2048:3072])
            nc.scalar.dma_start(tgt_c[3][:], tgt[:, 3072:4096])

            # V'^T tiles with denominator ones-column: [k-token, kt, c|1].
            # The column write is strided; gpsimd memset crashes the exec unit
            # on strided APs, DVE handles it.
            vta = big.tile([C, KT, C + 1], F16)
            nc.vector.memset(vta[:, :, C : C + 1], 1.0)
            zero_t = consts.tile([C, 1], F32)
            nc.vector.memset(zero_t[:], 0.0)

            # PE warm-up: dummy matmuls with no DMA deps ramp the HAM clock
            # while the input DMAs are in flight.
            for wi in range(6):
                warm_ps = psum.tile([C, 512], F32, tag="ps_big", bufs=2, name=f"warm_{wi}")
                nc.tensor.matmul(
                    warm_ps[:], warm_src[:, 0:128], warm_src[:], start=True, stop=True,
                )

            # ---- projections: all pieces share ONE spare PSUM bank, so they
            # never contend with the score-tile ring; each piece is a matmul
            # plus a DVE convert, self-serialized through the bank.
            k_c = [big.tile([C, 1024], F16, name=f"k_c{j}") for j in range(4)]

            def emit_kv(piece):
                kind, idx = piece[0], int(piece[1:])
                if kind == "k":
                    j, h = divmod(idx, 2)
                    kvp = psum_kv.tile([C, 512], F32, tag="kv", bufs=1, name=f"kp{idx}")
                    nc.tensor.matmul(
                        kvp[:], mt_h, tgt_c[j][:, h * 512 : (h + 1) * 512],
                        start=True, stop=True,
                    )
                    nc.vector.tensor_copy(k_c[j][:, h * 512 : (h + 1) * 512], kvp[:])
                else:
                    g = idx
                    kvp = psum_kv.tile([C, 2, C], F32, tag="kv", bufs=1, name=f"vp{idx}")
                    for i in range(2):
                        mt = g * 2 + i
                        nc.tensor.matmul(
                            kvp[:, i, :],
                            tgt_c[mt // 8][:, (mt % 8) * C : (mt % 8 + 1) * C],
                            wvo_h, start=True, stop=True,
                        )
                    nc.vector.tensor_copy(vta[:, g * 2 : (g + 1) * 2, 0:C], kvp[:])

            # ---- attention pipeline ----
            opsum = [
                psum_pv.tile([C, 3, 170], F32, name="opsum_a"),
                psum_pv.tile([C, 3, 170], F32, name="opsum_b"),
                psum_pv.tile([C, 2, 170], F32, name="opsum_c"),
            ]

            st_tiles = {}
            et_tiles = {}

            def emit_st(kt):
                st = psum.tile([C, NQ], F32, tag="ps_big", bufs=2, name=f"st_{kt}")
                for h in range(2):
                    nc.tensor.matmul(
                        st[:, h * 512 : (h + 1) * 512],
                        k_c[kt // 8][:, (kt % 8) * C : (kt % 8 + 1) * C],
                        srcq_sb[:, h * 512 : (h + 1) * 512],
                        start=True, stop=True,
                    )
                st_tiles[kt] = st

            def emit_exp(kt, split=False):
                et = ets.tile([C, NQ], F16, tag="et", name=f"et_{kt}")
                st = st_tiles.pop(kt)
                if split:
                    for h in range(2):
                        s = slice(h * 512, (h + 1) * 512)
                        nc.scalar.activation(
                            out=et[:, s], in_=st[:, s], func=AF.Exp,
                            bias=beta_sb[:, kt : kt + 1], scale=1.0,
                        )
                else:
                    nc.scalar.activation(
                        out=et[:], in_=st[:], func=AF.Exp,
                        bias=beta_sb[:, kt : kt + 1], scale=1.0,
                    )
                et_tiles[kt] = et

            def emit_pv(kt):
                et = et_tiles[kt]
                for j in range(QC):
                    t, idx = CHUNK_MAP[j]
                    nc.tensor.matmul(
                        opsum[t][:, idx, 0 : C + 1],
                        et[:, j * 128 : (j + 1) * 128],
                        vta[:, kt, :],
                        start=(kt == 0 and idx == 0),
                        stop=(kt == KT - 1),
                        skip_group_check=True,
                    )

            # ---- software-pipelined emission; kv pieces in deadline order.
            kv_order = [
                "v1", "k1", "v2", "v3", "k2", "v4", "v5", "k3", "v6", "v7",
                "k4", "v8", "v9", "k5", "v10", "v11", "k6", "v12", "v13",
                "k7", "v14", "v15",
            ]
            emit_kv("k0")
            emit_kv("v0")
            emit_st(0)
            emit_exp(0)
            nkv = 0
            for kt in range(1, KT):
                emit_st(kt)
                emit_exp(kt, split=(kt == KT - 1))
                if nkv < len(kv_order):
                    emit_kv(kv_order[nkv])
                    nkv += 1
                if kt >= 2:
                    emit_pv(kt - 2)
            emit_pv(KT - 2)
            emit_pv(KT - 1)

            # ---- epilogue: normalize [q, c] by the ones-column denominators;
            # norm ops split across Act (idle now) and DVE, DMA per half.
            recip_sb = big.tile([C, QC], F32)
            o_fin = big.tile([C, QC, C], F16)
            nc.vector.reciprocal(out=recip_sb[:, 0:3], in_=opsum[0][:, :, C])
            nc.vector.reciprocal(out=recip_sb[:, 3:6], in_=opsum[1][:, :, C])
            nc.vector.reciprocal(out=recip_sb[:, 6:8], in_=opsum[2][:, :, C])
            # chunks 0-3 on Act, 4-7 on DVE, concurrently; DMA per pair on
            # three rings so the generations don't serialize.
            rings = {0: nc.sync, 2: nc.scalar, 4: nc.sync, 6: nc.scalar}
            for j in (0, 4, 1, 5, 2, 6, 3, 7):
                t, idx = CHUNK_MAP[j]
                if j < 4:
                    nc.scalar.activation(
                        out=o_fin[:, j, :], in_=opsum[t][:, idx, 0:C],
                        func=AF.Identity, bias=zero_t[:],
                        scale=recip_sb[:, j : j + 1],
                    )
                else:
                    nc.vector.tensor_scalar(
                        out=o_fin[:, j, :], in0=opsum[t][:, idx, 0:C],
                        scalar1=recip_sb[:, j : j + 1], scalar2=None, op0=OP.mult,
                    )
                if j in (1, 3, 5, 7):
                    p = j - 1 if j < 4 else j - 1
                    pair = (j - 1) if True else p
                    ring = rings[j - 1]
                    ring.dma_start(out[:, j - 1 : j + 1, :], o_fin[:, j - 1 : j + 1, :])

    _split_excess_waits(nc)
    return nc


_NC_CACHE = None


def _get_nc():
    global _NC_CACHE
    if _NC_CACHE is None:
        _NC_CACHE = build_bass()
    return _NC_CACHE


def make_in_maps(source, target, wq, bq, wk, bk, wv, bv, wo, bo):
    source = np.asarray(source, dtype=np.float32).reshape(B, C, N)
    target = np.asarray(target, dtype=np.float32).reshape(B, C, N)
    wq, wk, wv, wo = (np.asarray(x, np.float32) for x in (wq, wk, wv, wo))
    bq, bk, bv, bo = (np.asarray(x, np.float32) for x in (bq, bk, bv, bo))
    scale = np.float32(1.0 / np.sqrt(C))

    M = (wq.T @ wk) * scale                 # [c_src, c_tgt]
    Wvo = wo @ wv                            # [c_out, c_tgt]
    mw_v = np.ascontiguousarray(
        np.stack([M.T, Wvo.T], axis=1).astype(np.float16)
    )                                        # [ct, 2, c]

    src16 = source.astype(np.float16)
    tgt16 = target.astype(np.float16)

    in_maps = []
    bet3_b = []
    for b in range(B):
        beta = ((bq @ (wk @ target[b])) + np.float32(bq @ bk)) * scale  # [N]
        bvec = beta.reshape(KT, 128).T.astype(np.float32)               # [128, KT]
        b1p = np.float32(B1) + np.float32(S1) * bvec
        w1p = np.float32(W1) + np.float32(S1) * bvec
        bet3_b.append(np.ascontiguousarray(
            np.stack([bvec, b1p, w1p], axis=1).astype(np.float32)
        ))                                                              # [128, 3, KT]

    for core in range(NCORES):
        b, qs = divmod(core, QSHARDS)
        in_maps.append({
            "srcq": np.ascontiguousarray(src16[b, :, qs * NQ : (qs + 1) * NQ]),
            "tgt": np.ascontiguousarray(tgt16[b]),
            "mw": mw_v,
            "bet3": bet3_b[b],
        })
    return in_maps


def kernel(source, target, wq, bq, wk, bk, wv, bv, wo, bo):
    nc = _get_nc()
    in_maps = make_in_maps(source, target, wq, bq, wk, bk, wv, bv, wo, bo)
    res = run_bass_kernel_spmd(nc, in_maps, core_ids=list(range(NCORES)))
    bvo = (np.asarray(wo, np.float32) @ np.asarray(bv, np.float32)
           + np.asarray(bo, np.float32))                                # [C]
    full = np.empty((B, C, N), dtype=np.float32)
    for core in range(NCORES):
        b, qs = divmod(core, QSHARDS)
        o = np.asarray(res.results[core]["out"], np.float32)            # [p, j, co]
        full[b, :, qs * NQ : (qs + 1) * NQ] = (
            o.transpose(2, 1, 0).reshape(C, NQ) + bvo[:, None]
        )
    return full.reshape(B, C, D, H, W)
